# revision 4
# baseline (speedup 1.0000x reference)
"""Relational (per-edge-type) multi-head attention on 8 Trainium2 NeuronCores.

Reference computation (per batch b, head h):
    for edge type e in 0..3:
        q_e = h @ Wq[e] + bq[e]   (head-sliced, scaled by 1/sqrt(d_k))
        k_e = h @ Wk[e] + bk[e]
        raw_e = q_e @ k_e.T
    s[n, m]   = raw_{adj[n,m]-1}[n, m]   if adj[n,m] in 1..4 else -1e9
    out       = softmax(s, axis=-1)      -> [heads, B, N, N]

Sharding: 8 cores = 4 batches x 2 head-groups (4 heads each). Each core
computes its (batch, head-group) block of the output: [4, 1024, 1024].

Per-core device pipeline:
  - PE: qT/kT projections per edge (K=256 contraction, weights pre-sliced
    and pre-scaled on host), then per-(head, n-tile, m-chunk, edge) score
    matmuls (K=64) into 4 PSUM banks (one per edge).
  - DVE: per-position selection of the right edge's score with a chain of
    copy_predicated ops driven by (adj == e+1) masks; positions with
    adj == 0 are initialized to -1e9.
  - ACT: exp with fused row-sum (softmax numerator + denominator; scores
    are O(1) so no row-max subtraction is needed), final 1/denom scaling.
"""

import math
import os
import sys
from contextlib import ExitStack

import numpy as np

sys.path.insert(0, os.path.dirname(os.path.abspath(__file__)))

import concourse.bass as bass
import concourse.tile as tile
from concourse import mybir
from concourse.bass_utils import run_bass_kernel_spmd

# -- problem constants (hardcoded; kernel.py must be self-contained) --
E, B, N, F_IN, F_OUT, NH = 4, 4, 1024, 256, 512, 8
D_K = F_OUT // NH            # 64
N_CORES = 8
HEADS_PER_CORE = NH // 2     # 4 heads per core (2 head groups)
GROUP_F = HEADS_PER_CORE * D_K   # 256 output features per head group
P = 128                      # partitions
NTILES = N // P              # 8 row tiles
NEG = -1.0e9

_FP32 = mybir.dt.float32
_I32 = mybir.dt.int32


def _split_multi_waits(bir_bytes: bytes) -> bytes:
    """Rewrite BIR so each instruction carries at most one sync wait.

    The walrus build in this container rejects instructions with >1 sync
    wait. Extra waits are hoisted onto EventSemaphore instructions inserted
    immediately before the owner on the same engine (program order on the
    sequencer preserves semantics).
    """
    import json

    d = json.loads(bir_bytes)
    for fn in d["functions"]:
        for blk in fn["blocks"]:
            out = []
            for ins in blk["instructions"]:
                si = ins.get("sync_info")
                waits = (si or {}).get("on_wait") or []
                if len(waits) > 1:
                    for j, w in enumerate(waits[:-1]):
                        out.append(
                            {
                                "debug": ins.get("debug", 0),
                                "engine": ins["engine"],
                                "ins": [],
                                "outs": [],
                                "name": f"{ins['name']}_ws{j}",
                                "opcode": "EventSemaphore",
                                "sync_info": {"on_update": [], "on_wait": [w]},
                            }
                        )
                    si["on_wait"] = [waits[-1]]
                out.append(ins)
            blk["instructions"] = out
    return json.dumps(d).encode()


def build_program() -> bass.Bass:
    nc = bass.Bass()

    hT = nc.dram_tensor("hT", [F_IN, N], _FP32, kind="ExternalInput")
    adj = nc.dram_tensor("adj", [N, N], _I32, kind="ExternalInput")
    # weights: [e*4 + src*2 + ktile, 128, GROUP_F]  (src: 0=q, 1=k)
    w = nc.dram_tensor("w", [E * 4, P, GROUP_F], _FP32, kind="ExternalInput")
    # biases: [e*4 + src*2 + mtile, 128, 1]
    bvec = nc.dram_tensor("bvec", [E * 4, P, 1], _FP32, kind="ExternalInput")
    o = nc.dram_tensor("o", [HEADS_PER_CORE, N, N], _FP32, kind="ExternalOutput")

    KT = F_IN // P       # 2 k-tiles for projections
    MT = GROUP_F // P    # 2 m-tiles of projected features per head group
    NCHUNK = 512         # matmul free-dim chunk
    NCH = N // NCHUNK    # 2 chunks per row

    with tile.TileContext(nc) as tc:
        with ExitStack() as ctx:
            persist = ctx.enter_context(tc.tile_pool(name="persist", bufs=1))
            work = ctx.enter_context(tc.tile_pool(name="work", bufs=2))
            outp = ctx.enter_context(tc.tile_pool(name="outp", bufs=3))
            psp = ctx.enter_context(tc.tile_pool(name="psp", bufs=2, space="PSUM"))

            # ---- one-time loads ----
            hT_sb = []
            for kt in range(KT):
                t = persist.tile([P, N], _FP32, tag=f"hT{kt}")
                nc.sync.dma_start(out=t, in_=hT[kt * P:(kt + 1) * P, :])
                hT_sb.append(t)

            w_sb = {}
            b_sb = {}
            for i in range(E * 4):
                wt = persist.tile([P, GROUP_F], _FP32, tag=f"w{i}")
                nc.sync.dma_start(out=wt, in_=w[i])
                w_sb[i] = wt
                bt = persist.tile([P, 1], _FP32, tag=f"b{i}")
                nc.sync.dma_start(out=bt, in_=bvec[i])
                b_sb[i] = bt

            # ---- projections: qT/kT [GROUP_F, N] per edge, bias added ----
            # qk_sb[(e, src, mtile)] = [128, N] tile; partition = feature d
            qk_sb = {}
            evac_cnt = 0
            for e in range(E):
                for src in range(2):  # 0 = q, 1 = k
                    for mt in range(MT):
                        dst = persist.tile([P, N], _FP32, tag=f"qk{e}_{src}_{mt}")
                        qk_sb[(e, src, mt)] = dst
                        # shares the psmain slots (PSUM fits only 2 x 4 banks)
                        ps = psp.tile([P, E, NCHUNK], _FP32, tag="psmain")
                        for chunk in range(NCH):
                            for kt in range(KT):
                                nc.tensor.matmul(
                                    ps[:, chunk, :],
                                    lhsT=w_sb[e * 4 + src * 2 + kt][
                                        :, mt * P:(mt + 1) * P
                                    ],
                                    rhs=hT_sb[kt][
                                        :, chunk * NCHUNK:(chunk + 1) * NCHUNK
                                    ],
                                    start=(kt == 0),
                                    stop=(kt == KT - 1),
                                )
                        # evacuate + bias add (alternate DVE / ACT)
                        bias_ap = b_sb[e * 4 + src * 2 + mt]
                        ps2 = ps[:, 0:NCH, :].rearrange("p a b -> p (a b)")
                        if evac_cnt % 2 == 0:
                            nc.vector.tensor_scalar(
                                dst[:, :], ps2, bias_ap, None, mybir.AluOpType.add,
                            )
                        else:
                            nc.scalar.add(dst[:, :], ps2, bias_ap)
                        evac_cnt += 1

            # ---- main loop: n-tiles x heads x m-chunks ----
            for nt in range(NTILES):
                adj_sb = work.tile([P, N], _I32, tag="adj")
                nc.sync.dma_start(out=adj_sb, in_=adj[nt * P:(nt + 1) * P, :])

                # per-edge masks (adj == e+1), reused by the 4 heads
                masks = []
                for e in range(E):
                    m = work.tile([P, N], mybir.dt.int8, tag=f"mask{e}")
                    nc.vector.tensor_scalar(
                        m, adj_sb, float(e + 1), None, mybir.AluOpType.is_equal
                    )
                    masks.append(m)

                for hd in range(HEADS_PER_CORE):
                    mt = hd // 2
                    pbase = (hd % 2) * D_K

                    # selection target: -1e9 where adj==0, else overwritten
                    sel = work.tile([P, N], _FP32, tag="sel")
                    nc.vector.tensor_scalar(
                        sel, adj_sb, 0.0, NEG,
                        mybir.AluOpType.is_equal, mybir.AluOpType.mult,
                    )

                    for chunk in range(NCH):
                        ps = psp.tile([P, E, NCHUNK], _FP32, tag="psmain")
                        for e in range(E):
                            nc.tensor.matmul(
                                ps[:, e, :],
                                lhsT=qk_sb[(e, 0, mt)][
                                    pbase:pbase + D_K, nt * P:(nt + 1) * P
                                ],
                                rhs=qk_sb[(e, 1, mt)][
                                    pbase:pbase + D_K,
                                    chunk * NCHUNK:(chunk + 1) * NCHUNK,
                                ],
                                start=True,
                                stop=True,
                            )
                        cs = slice(chunk * NCHUNK, (chunk + 1) * NCHUNK)
                        for e in range(E):
                            nc.vector.copy_predicated(
                                sel[:, cs], masks[e][:, cs], ps[:, e, :]
                            )

                    # softmax without row-max (scores are O(1))
                    numer = work.tile([P, N], _FP32, tag="numer")
                    den = work.tile([P, 1], _FP32, tag="den")
                    nc.scalar.activation(
                        numer, sel, mybir.ActivationFunctionType.Exp,
                        accum_out=den,
                    )
                    rden = work.tile([P, 1], _FP32, tag="rden")
                    nc.vector.reciprocal(rden, den)

                    out_sb = outp.tile([P, N], _FP32, tag="out")
                    nc.scalar.mul(out_sb, numer, rden)
                    nc.sync.dma_start(
                        out=o[hd, nt * P:(nt + 1) * P, :], in_=out_sb
                    )

    # wrap serialization with the wait-splitting fix
    orig = nc.to_json_bytes
    nc.to_json_bytes = lambda *a, **k: _split_multi_waits(orig(*a, **k))
    return nc


_CACHED_NC = None


def _get_nc():
    global _CACHED_NC
    if _CACHED_NC is None:
        _CACHED_NC = build_program()
    return _CACHED_NC


def make_in_maps(h, adj_mat, Wq, bq, Wk, bk):
    """Host-side shard/prep: slice per (batch, head-group), fold the
    1/sqrt(d_k) scale into the q-side weights, pre-transpose h."""
    scale = 1.0 / math.sqrt(D_K)
    Wq_s = (Wq * scale).astype(np.float32)
    bq_s = (bq * scale).astype(np.float32)
    in_maps = []
    for c in range(N_CORES):
        b, g = c // 2, c % 2
        hT = np.ascontiguousarray(h[b].T.astype(np.float32))
        adj = np.ascontiguousarray(adj_mat[b].astype(np.int32))
        w = np.empty((E * 4, P, GROUP_F), np.float32)
        bv = np.empty((E * 4, P, 1), np.float32)
        gsl = slice(g * GROUP_F, (g + 1) * GROUP_F)
        for e in range(E):
            for src, (W, bias) in enumerate(((Wq_s, bq_s), (Wk, bk))):
                for kt in range(F_IN // P):
                    w[e * 4 + src * 2 + kt] = W[e, kt * P:(kt + 1) * P, gsl]
                for mt in range(GROUP_F // P):
                    bv[e * 4 + src * 2 + mt, :, 0] = bias[
                        e, g * GROUP_F + mt * P: g * GROUP_F + (mt + 1) * P
                    ]
        in_maps.append({"hT": hT, "adj": adj, "w": w,
                        "bvec": np.ascontiguousarray(bv)})
    return in_maps


def kernel(h, adj_mat, Wq, bq, Wk, bk, _trace=False, _trace_kwargs=None):
    h = np.asarray(h)
    adj_mat = np.asarray(adj_mat)
    Wq, bq, Wk, bk = (np.asarray(x) for x in (Wq, bq, Wk, bk))

    nc = _get_nc()
    in_maps = make_in_maps(h, adj_mat, Wq, bq, Wk, bk)
    kwargs = {}
    if _trace:
        kwargs = dict(trace=True, **(_trace_kwargs or {}))
    res = run_bass_kernel_spmd(nc, in_maps, core_ids=list(range(N_CORES)),
                               **kwargs)
    out = np.empty((NH, B, N, N), np.float32)
    for c in range(N_CORES):
        b, g = c // 2, c % 2
        out[g * HEADS_PER_CORE:(g + 1) * HEADS_PER_CORE, b] = res.results[c]["o"]
    if _trace:
        return out, res
    return out
